# revision 2
# baseline (speedup 1.0000x reference)
"""Multi-head attention (B=4, S=2048, E=1024, H=16, D=64) on 8 trn2 cores.

Sharding: 2D (batch x head-group). Core c handles batch b = c//2 and head
group g = c%2 (8 heads = 512 feature dims). Each core computes a full
[S, E] partial of the output projection for its batch; the host sums the
two group partials per batch and adds the bias.

Per-core device kernel (all fp16/bf16 matmuls, fp32 PSUM accumulation):
  qT = (Wq_loc @ X_q^T)      [512, 2048]  (stored transposed, fp16)
  kT = (Wk_loc @ X_k^T)      [512, 2048]
  v  = X_v @ Wv_loc^T        [2048, 512]  (natural layout + ones column, bf16)
  per head h, per qq-tile (1024), per kk-chunk (128):
    scores^T chunk = kT_h_chunk.T @ qT_h   -> PSUM [128, 1024] f32
    P^T = exp(scores^T)  (ScalarE, no max subtraction: max score ~76,
                          exp fits fp32/bf16 range)   -> SBUF bf16
    U  += v_aug_chunk.T @ P^T  -> PSUM [65, 1024] f32  (row 64 = softmax denom)
  attnout^T = U[0:64] * broadcast(1/U[64])  -> SBUF fp16
  y = attnout^T.T @ Wo_loc^T  -> [2048, 1024] f32 partial
"""

from contextlib import ExitStack

import numpy as np

S = 2048
E = 1024
F = 512          # local feature dims (8 heads x 64)
HL = 8           # heads per core
D = 64
B = 4
H = 16
NCORES = 8

_CACHE = {}


def build_nc(reps: int = 1):
    import concourse.tile as tile
    from concourse import bacc, mybir

    F16 = mybir.dt.float16
    BF16 = mybir.dt.bfloat16
    F32 = mybir.dt.float32
    EXP = mybir.ActivationFunctionType.Exp

    nc = bacc.Bacc(
        "TRN2",
        target_bir_lowering=False,
        debug=False,
        enable_asserts=False,
        num_devices=NCORES,
    )

    xq_d = nc.dram_tensor("xq", [E, S], F16, kind="ExternalInput").ap()
    xk_d = nc.dram_tensor("xk", [E, S], F16, kind="ExternalInput").ap()
    xv_d = nc.dram_tensor("xv", [E, S], F16, kind="ExternalInput").ap()
    wq_d = nc.dram_tensor("wq", [E, F], F16, kind="ExternalInput").ap()
    wk_d = nc.dram_tensor("wk", [E, F], F16, kind="ExternalInput").ap()
    wv_d = nc.dram_tensor("wv", [E, F], F16, kind="ExternalInput").ap()
    wo_d = nc.dram_tensor("wo", [F, E], F16, kind="ExternalInput").ap()
    y_d = nc.dram_tensor("y", [S, E], F32, kind="ExternalOutput").ap()

    with tile.TileContext(nc) as tc, ExitStack() as ctx:
        persist = ctx.enter_context(tc.tile_pool(name="persist", bufs=1))
        xpool = ctx.enter_context(tc.tile_pool(name="xpool", bufs=10))
        ppool = ctx.enter_context(tc.tile_pool(name="ppool", bufs=3))
        ypool = ctx.enter_context(tc.tile_pool(name="ypool", bufs=2))
        smpool = ctx.enter_context(tc.tile_pool(name="smpool", bufs=2))
        ps_s = ctx.enter_context(tc.tile_pool(name="ps_s", bufs=2, space="PSUM"))
        ps_u = ctx.enter_context(tc.tile_pool(name="ps_u", bufs=2, space="PSUM"))

        def body(iv):
            # ---------------- weights ----------------
            def load_w(dram, pfx, width):
                tiles = []
                nchunks = dram.shape[0] // 128
                for i in range(nchunks):
                    t = persist.tile([128, width], F16, tag=f"{pfx}{i}",
                                     name=f"{pfx}_sb{i}")
                    nc.sync.dma_start(t[:], dram[i * 128:(i + 1) * 128, :])
                    tiles.append(t)
                return tiles

            wq_sb = load_w(wq_d, "wq", F)
            wk_sb = load_w(wk_d, "wk", F)
            wv_sb = load_w(wv_d, "wv", F)
            wo_sb = load_w(wo_d, "wo", E)

            # v with ones column: v_sb[p, tc, h, d] = v[tc*128+p, h*64+d],
            # d=64 column stays 1.0 (softmax denominator trick)
            v_sb = persist.tile([128, 16, HL, D + 1], BF16, tag="v_sb",
                                name="v_sb")
            nc.vector.memset(v_sb[:], 1.0)

            # ---------------- V projection (natural layout) ----------------
            xv_sb = []
            for eci in range(8):
                t = xpool.tile([128, S], F16, tag="x", name=f"xv_sb{eci}")
                nc.sync.dma_start(t[:], xv_d[eci * 128:(eci + 1) * 128, :])
                xv_sb.append(t)
            for tci in range(16):
                vp = ps_u.tile([128, F], F32, tag="u", name=f"v_ps{tci}")
                for eci in range(8):
                    nc.tensor.matmul(
                        vp[:],
                        lhsT=xv_sb[eci][:, tci * 128:(tci + 1) * 128],
                        rhs=wv_sb[eci][:],
                        start=(eci == 0),
                        stop=(eci == 7),
                    )
                for h in range(HL):
                    nc.vector.tensor_copy(v_sb[:, tci, h, 0:D],
                                          vp[:, h * D:(h + 1) * D])

            # ---------------- Q/K projections (transposed layout) ----------
            def proj_T(x_d, w_sb, pfx):
                x_sb = []
                for eci in range(8):
                    t = xpool.tile([128, S], F16, tag="x", name=f"x{pfx}{eci}")
                    nc.sync.dma_start(t[:], x_d[eci * 128:(eci + 1) * 128, :])
                    x_sb.append(t)
                outs = []
                for oci in range(4):
                    ot = persist.tile([128, S], F16, tag=f"{pfx}T{oci}",
                                      name=f"{pfx}T_sb{oci}")
                    for half in range(2):
                        pp = ps_s.tile([128, 1024], F32, tag="s",
                                       name=f"{pfx}p{oci}_{half}")
                        for eci in range(8):
                            for nb in range(2):
                                col = half * 1024 + nb * 512
                                nc.tensor.matmul(
                                    pp[:, nb * 512:(nb + 1) * 512],
                                    lhsT=w_sb[eci][:, oci * 128:(oci + 1) * 128],
                                    rhs=x_sb[eci][:, col:col + 512],
                                    start=(eci == 0),
                                    stop=(eci == 7),
                                )
                        nc.vector.tensor_copy(
                            ot[:, half * 1024:(half + 1) * 1024], pp[:])
                    outs.append(ot)
                return outs

            qT_sb = proj_T(xq_d, wq_sb, "q")
            kT_sb = proj_T(xk_d, wk_sb, "k")

            # attnout^T storage
            aT_sb = [persist.tile([128, S], F16, tag=f"aT{i}", name=f"aT_sb{i}")
                     for i in range(4)]

            # ---------------- attention ----------------
            import concourse.bass as bass

            for h in range(HL):
                ch, hh = h // 2, h % 2
                p0, p1 = hh * 64, hh * 64 + 64
                for qt in range(2):
                    U = ps_u.tile([65, 1024], F32, tag="u", name=f"U{h}_{qt}")
                    prev = None

                    def av(kk, pt):
                        for nb in range(2):
                            nc.tensor.matmul(
                                U[:, nb * 512:(nb + 1) * 512],
                                lhsT=v_sb[:, kk, h, :],
                                rhs=pt[:, nb * 512:(nb + 1) * 512],
                                start=(kk == 0),
                                stop=(kk == 15),
                            )

                    for kk in range(16):
                        sc = ps_s.tile([128, 1024], F32, tag="s",
                                       name=f"sc{h}_{qt}_{kk}")
                        for nb in range(2):
                            qcol = qt * 1024 + nb * 512
                            nc.tensor.matmul(
                                sc[:, nb * 512:(nb + 1) * 512],
                                lhsT=kT_sb[ch][p0:p1, kk * 128:(kk + 1) * 128],
                                rhs=qT_sb[ch][p0:p1, qcol:qcol + 512],
                                start=True,
                                stop=True,
                            )
                        # AV of previous chunk emitted before exp(kk) so the
                        # PE never waits on ACT for the next scores chunk
                        if prev is not None:
                            av(*prev)
                        pt = ppool.tile([128, 1024], BF16, tag="p",
                                        name=f"p{h}_{qt}_{kk}")
                        nc.scalar.activation(pt[:], sc[:], EXP)
                        prev = (kk, pt)
                    av(*prev)

                    # normalize: aT = U[0:64] / U[64]
                    rcp = smpool.tile([1, 1024], F32, tag="rcp",
                                      name=f"rcp{h}_{qt}")
                    nc.vector.reciprocal(rcp[:], U[64:65, :])
                    bc = smpool.tile([64, 1024], F32, tag="bc",
                                     name=f"bc{h}_{qt}")
                    nc.gpsimd.partition_broadcast(bc[:], rcp[:])
                    nc.vector.tensor_mul(
                        aT_sb[ch][p0:p1, qt * 1024:(qt + 1) * 1024],
                        U[0:64, :], bc[:])

            # ---------------- output projection ----------------
            for tci in range(16):
                yp = ps_u.tile([128, 1024], F32, tag="u", name=f"y_ps{tci}")
                for fc in range(4):
                    for nb in range(2):
                        nc.tensor.matmul(
                            yp[:, nb * 512:(nb + 1) * 512],
                            lhsT=aT_sb[fc][:, tci * 128:(tci + 1) * 128],
                            rhs=wo_sb[fc][:, nb * 512:(nb + 1) * 512],
                            start=(fc == 0),
                            stop=(fc == 3),
                        )
                ysb = ypool.tile([128, 1024], F32, tag="y", name=f"y_sb{tci}")
                nc.vector.tensor_copy(ysb[:], yp[:])
                nc.sync.dma_start(y_d[tci * 128:(tci + 1) * 128, :], ysb[:])

        if reps == 1:
            body(0)
        else:
            with tc.For_i(0, reps, 1) as iv:
                body(iv)

    nc.compile()
    return nc


def make_in_maps(Q, K, V, Wq, Wk, Wv, Wo):
    """Shard + lay out full inputs for the 8 cores."""
    Q = np.asarray(Q, dtype=np.float32)
    K = np.asarray(K, dtype=np.float32)
    V = np.asarray(V, dtype=np.float32)
    Wq = np.asarray(Wq, dtype=np.float32)
    Wk = np.asarray(Wk, dtype=np.float32)
    Wv = np.asarray(Wv, dtype=np.float32)
    Wo = np.asarray(Wo, dtype=np.float32)

    in_maps = []
    for c in range(NCORES):
        b, g = c // 2, c % 2
        rows = slice(g * F, (g + 1) * F)
        in_maps.append({
            "xq": np.ascontiguousarray(Q[b].T).astype(np.float16),
            "xk": np.ascontiguousarray(K[b].T).astype(np.float16),
            "xv": np.ascontiguousarray(V[b].T).astype(np.float16),
            "wq": np.ascontiguousarray(Wq[rows, :].T).astype(np.float16),
            "wk": np.ascontiguousarray(Wk[rows, :].T).astype(np.float16),
            "wv": np.ascontiguousarray(Wv[rows, :].T).astype(np.float16),
            "wo": np.ascontiguousarray(Wo[:, rows].T).astype(np.float16),
        })
    return in_maps


def combine(results, bo):
    """Sum per-core partials + bias -> full [B, S, E] output."""
    bo = np.asarray(bo, dtype=np.float32)
    y = np.zeros((B, S, E), dtype=np.float32)
    for c in range(NCORES):
        y[c // 2] += results[c]["y"]
    y += bo[None, None, :]
    return y


def kernel(Q, K, V, Wq, Wk, Wv, Wo, bo):
    from concourse.bass_utils import run_bass_kernel_spmd

    if "nc" not in _CACHE:
        _CACHE["nc"] = build_nc(reps=1)
    nc = _CACHE["nc"]
    in_maps = make_in_maps(Q, K, V, Wq, Wk, Wv, Wo)
    res = run_bass_kernel_spmd(nc, in_maps, core_ids=list(range(NCORES)))
    return combine(res.results, bo)
